# revision 1
# baseline (speedup 1.0000x reference)
"""Bass/Tile TRN2 kernel for nn_EnhancedMinkConv2D (sparse 3x3 convs + SE attention).

Strategy (8 NeuronCores, SPMD):
  - Shard the N=300000 active sites across the 8 cores (37500 each, padded
    to a superchunk multiple). Replicate the feature table (fp16, with one
    all-zero dummy row) and all weights on every core.
  - Per core, per superchunk of S sites: indirect DMA gathers (one row per
    partition per instruction - the only form the HW DGE supports) fetch all
    9 neighbor rows per site from the fp16 table for each dilation. Missing
    neighbors (-1) are remapped host-side to a dummy zero row, which
    reproduces the reference masking exactly for the linear paths; path 2's
    nonlinear h gets an exact constant correction (em9) via a mask matmul.
  - Gathered tiles are site-major [128 sites, ch]; PE transposes flip them
    to channel-major [ch, sites] so the convs become plain GEMMs with the
    contraction on partitions.
  - Path 2's bottleneck h = relu(bn(f @ W2a)) is recomputed on the gathered
    data (blockdiag-packed to use the full array) instead of gathering a
    separate h table - this trades cheap PE flops for 1/3 of the gather
    traffic.
  - ms = [feat1|feat3|feat2] (channel-permuted; all downstream 192-dim
    weights are permuted host-side to match) is reduced (running max) and
    stored to DRAM in fp16.
  - A [192] AllReduce(max) + the tiny SE MLP run on every core; the
    attention vector is folded into the fusion weights (ms*attn @ Wf ==
    ms @ (attn[:,None]*Wf)).
  - Pass 2 streams ms back, runs the fusion GEMM + BN/ReLU, transposes back
    to site-major and writes the output rows.
"""

import numpy as np

P = 128
JB = 4  # j-columns per block (block = 512 sites)
BLK = P * JB


def build_kernel(NT, nlp, S, n_cores, DUM):
    """Build the Bass module. NT = table rows, nlp = padded sites per core,
    S = sites per superchunk (must be P*JS with JS % JB == 0)."""
    import concourse.bacc as bacc
    from concourse import bass, mybir, tile
    from concourse.bass import IndirectOffsetOnAxis
    from concourse.masks import make_identity

    JS = S // P
    assert JS % JB == 0 and nlp % S == 0
    n_sc = nlp // S
    n_blk_sc = JS // JB
    n_blocks = n_sc * n_blk_sc

    f16 = mybir.dt.float16
    f32 = mybir.dt.float32
    i32 = mybir.dt.int32
    Relu = mybir.ActivationFunctionType.Relu
    Sigmoid = mybir.ActivationFunctionType.Sigmoid

    nc = bacc.Bacc("TRN2", target_bir_lowering=False, debug=False)

    def din(name, shape, dt):
        return nc.dram_tensor(name, shape, dt, kind="ExternalInput")

    ftab = din("ftab", [NT, 64], f16)
    nbr1 = din("nbr1", [9, nlp], i32)
    nbr2 = din("nbr2", [9, nlp], i32)
    w1p_d = din("w1p", [4 * 128, 64], f16)   # vstacked pairs of W1[k]
    w1s_d = din("w1s", [64, 64], f16)        # W1[8]
    w3p_d = din("w3p", [4 * 128, 64], f16)
    w3s_d = din("w3s", [64, 64], f16)
    w2abd_d = din("w2abd", [128, 64], f16)   # blockdiag(W2a, W2a)
    w2as_d = din("w2as", [64, 32], f16)      # W2a
    w2bs_d = din("w2bs", [2 * 128, 64], f16)  # vstack(W2b[0:4]), vstack(W2b[4:8])
    w2b8_d = din("w2b8", [32, 64], f16)      # W2b[8]
    wfa_d = din("wfa", [128, 64], f32)       # Wf permuted rows 0:128
    wfb_d = din("wfb", [64, 64], f32)        # Wf permuted rows 128:192
    a1wA_d = din("a1wA", [128, 16], f32)
    a1wB_d = din("a1wB", [64, 16], f32)
    a1b_d = din("a1b", [16, 1], f32)
    a2wA_d = din("a2wA", [16, 128], f32)
    a2wB_d = din("a2wB", [16, 64], f32)
    a2bA_d = din("a2bA", [128, 1], f32)
    a2bB_d = din("a2bB", [64, 1], f32)
    bn13s_d = din("bn13s", [128, 1], f32)
    bn13b_d = din("bn13b", [128, 1], f32)
    bn2a4s_d = din("bn2a4s", [128, 1], f32)
    bn2a4b_d = din("bn2a4b", [128, 1], f32)
    bn2bs_d = din("bn2bs", [64, 1], f32)
    bn2bb_d = din("bn2bb", [64, 1], f32)
    bnfs_d = din("bnfs", [64, 1], f32)
    bnfb_d = din("bnfb", [64, 1], f32)
    em9_d = din("em9", [9, 64], f16)   # -(relu(bn2a_b) @ W2b[k])
    floc_d = din("floc", [nlp, 64], f16)  # this core's feature rows (padded)

    out_d = nc.dram_tensor("out", [nlp, 64], f32, kind="ExternalOutput")

    msA_d = nc.dram_tensor("msA_d", [n_blocks * 128, BLK], f16)
    msB_d = nc.dram_tensor("msB_d", [n_blocks * 64, BLK], f16)
    ccin = nc.dram_tensor("ccin", [1, 192], f32)
    ccout = nc.dram_tensor("ccout", [1, 192], f32)

    with tile.TileContext(nc) as tc:
        with tc.tile_pool(name="const", bufs=1) as cp:

            _cn = [0]

            def cload(dram_ap, shape, dt, name_='w'):
                _cn[0] += 1
                nm = 'c%d_%s' % (_cn[0], name_)
                t = cp.tile(shape, dt, name=nm, tag=nm)
                nc.sync.dma_start(out=t[:], in_=dram_ap)
                return t

            w1p_t = [cload(w1p_d[j * 128:(j + 1) * 128, :], [128, 64], f16,
                           'w1p%d' % j) for j in range(4)]
            w3p_t = [cload(w3p_d[j * 128:(j + 1) * 128, :], [128, 64], f16,
                           'w3p%d' % j) for j in range(4)]
            w1s_t = cload(w1s_d[:, :], [64, 64], f16)
            w3s_t = cload(w3s_d[:, :], [64, 64], f16)
            w2abd_t = cload(w2abd_d[:, :], [128, 64], f16)
            w2as_t = cload(w2as_d[:, :], [64, 32], f16)
            w2bs_t = [cload(w2bs_d[j * 128:(j + 1) * 128, :], [128, 64], f16,
                            'w2bs%d' % j) for j in range(2)]
            w2b8_t = cload(w2b8_d[:, :], [32, 64], f16)
            wfa_t = cload(wfa_d[:, :], [128, 64], f32)
            wfb_t = cload(wfb_d[:, :], [64, 64], f32)
            a1wA_t = cload(a1wA_d[:, :], [128, 16], f32)
            a1wB_t = cload(a1wB_d[:, :], [64, 16], f32)
            a1b_t = cload(a1b_d[:, :], [16, 1], f32)
            a2wA_t = cload(a2wA_d[:, :], [16, 128], f32)
            a2wB_t = cload(a2wB_d[:, :], [16, 64], f32)
            a2bA_t = cload(a2bA_d[:, :], [128, 1], f32)
            a2bB_t = cload(a2bB_d[:, :], [64, 1], f32)
            bn13s_t = cload(bn13s_d[:, :], [128, 1], f32)
            bn13b_t = cload(bn13b_d[:, :], [128, 1], f32)
            bn2a4s_t = cload(bn2a4s_d[:, :], [128, 1], f32)
            bn2a4b_t = cload(bn2a4b_d[:, :], [128, 1], f32)
            bn2bs_t = cload(bn2bs_d[:, :], [64, 1], f32)
            bn2bb_t = cload(bn2bb_d[:, :], [64, 1], f32)
            bnfs_t = cload(bnfs_d[:, :], [64, 1], f32)
            bnfb_t = cload(bnfb_d[:, :], [64, 1], f32)
            em9_t = cload(em9_d[:, :], [9, 64], f16)

            identH = cp.tile([128, 128], f16)
            make_identity(nc, identH[:])
            identF = cp.tile([64, 64], f32)
            make_identity(nc, identF[:])

            rmA = cp.tile([128, n_blocks], f32)
            rmB = cp.tile([64, n_blocks], f32)

            # ---------------- pass 1 ----------------
            with tc.tile_pool(name="gp", bufs=2) as gp, \
                 tc.tile_pool(name="ip", bufs=2) as ip, \
                 tc.tile_pool(name="tp", bufs=2, space="PSUM") as tp, \
                 tc.tile_pool(name="chp", bufs=2) as chp, \
                 tc.tile_pool(name="ftp", bufs=2, space="PSUM") as ftp, \
                 tc.tile_pool(name="ap", bufs=1, space="PSUM") as ap, \
                 tc.tile_pool(name="hp", bufs=2) as hp, \
                 tc.tile_pool(name="msp", bufs=2) as msp:
                for sc in range(n_sc):
                    gs = []
                    for nbr, tg in ((nbr1, "1"), (nbr2, "2")):
                        idx = ip.tile([128, JS, 9], i32, tag="idx" + tg)
                        nc.sync.dma_start(
                            out=idx[:],
                            in_=nbr[:, sc * S:(sc + 1) * S].rearrange(
                                "k (p j) -> p j k", p=P))
                        g = gp.tile([128, JS, 9, 64], f16, tag="g" + tg)
                        # HW indirect DMA only supports one gathered row per
                        # partition per instruction (offset [P,1], dest [P,D]).
                        # k=4 is the center tap (nbr[4,n] == n for both
                        # dilations): stream it from the core's own slice via
                        # HWDGE instead of burning a Pool-engine gather.
                        for j in range(JS):
                            for k in range(9):
                                if k == 4:
                                    continue
                                nc.gpsimd.indirect_dma_start(
                                    out=g[:, j, k, :], out_offset=None,
                                    in_=ftab[:, :],
                                    in_offset=IndirectOffsetOnAxis(
                                        ap=idx[:, j, k:k + 1], axis=0))
                        nc.sync.dma_start(
                            out=g[:, :, 4, :],
                            in_=floc_d[sc * S:(sc + 1) * S, :].rearrange(
                                "(p j) c -> p j c", p=P))
                        gs.append(g)
                        if tg == "1":
                            idx1s = idx
                    g1, g2 = gs
                    # site-major dummy mask for d1 (1.0 where neighbor missing)
                    msk1 = ip.tile([128, JS, 9], f16, tag="msk1")
                    nc.vector.tensor_scalar(
                        out=msk1[:], in0=idx1s[:], scalar1=DUM, scalar2=None,
                        op0=mybir.AluOpType.is_equal)

                    for b in range(n_blk_sc):
                        blk = sc * n_blk_sc + b
                        j0 = b * JB
                        # transpose gathered site-major tiles to channel-major
                        ch1, ch2, ch1s, ch2s = [], [], None, None
                        for g, chl in ((g1, ch1), (g2, ch2)):
                            for pr in range(4):
                                pt = tp.tile([128, BLK], f16, tag="tp")
                                for jj in range(JB):
                                    nc.tensor.transpose(
                                        out=pt[:, jj * 128:(jj + 1) * 128],
                                        in_=g[:, j0 + jj, 2 * pr:2 * pr + 2, :]
                                            .rearrange("p a c -> p (a c)"),
                                        identity=identH[:])
                                ch = chp.tile([128, BLK], f16, tag="ch", bufs=16)
                                nc.vector.tensor_copy(out=ch[:], in_=pt[:])
                                chl.append(ch)
                        for g, which in ((g1, "1"), (g2, "2")):
                            pt = tp.tile([64, BLK], f16, tag="tp")
                            for jj in range(JB):
                                nc.tensor.transpose(
                                    out=pt[:, jj * 128:(jj + 1) * 128],
                                    in_=g[:, j0 + jj, 8, :],
                                    identity=identH[:])
                            ch = chp.tile([64, BLK], f16, tag="chs", bufs=4)
                            nc.vector.tensor_copy(out=ch[:], in_=pt[:])
                            if which == "1":
                                ch1s = ch
                            else:
                                ch2s = ch

                        # paths 1 & 3: 9-pt convs, feat1 -> partitions 0:64,
                        # feat3 -> partitions 64:128 of one PSUM tile
                        ft13 = ftp.tile([128, BLK], f32, tag="ft13")
                        for pr in range(4):
                            nc.tensor.matmul(
                                out=ft13[0:64, :], lhsT=w1p_t[pr][:],
                                rhs=ch1[pr][:], start=(pr == 0), stop=False)
                        nc.tensor.matmul(
                            out=ft13[0:64, :], lhsT=w1s_t[:], rhs=ch1s[:],
                            start=False, stop=True)
                        for pr in range(4):
                            nc.tensor.matmul(
                                out=ft13[64:128, :], lhsT=w3p_t[pr][:],
                                rhs=ch2[pr][:], start=(pr == 0), stop=False)
                        nc.tensor.matmul(
                            out=ft13[64:128, :], lhsT=w3s_t[:], rhs=ch2s[:],
                            start=False, stop=True)

                        # path 2 stage A: h for all 9 ks (pairs via blockdiag)
                        psA = [ap.tile([128, BLK], f32, tag="apA%d" % i, name="psA%d" % i)
                               for i in range(2)]
                        psA2 = ap.tile([32, BLK], f32, tag="apB")
                        for pr in range(4):
                            nc.tensor.matmul(
                                out=psA[pr // 2][64 * (pr % 2):64 * (pr % 2) + 64, :],
                                lhsT=w2abd_t[:], rhs=ch1[pr][:],
                                start=True, stop=True)
                        nc.tensor.matmul(out=psA2[:], lhsT=w2as_t[:],
                                         rhs=ch1s[:], start=True, stop=True)
                        hA = [hp.tile([128, BLK], f16, tag="hA%d" % i, name="hA%d" % i)
                              for i in range(2)]
                        hA2 = hp.tile([32, BLK], f16, tag="hB")
                        for i in range(2):
                            nc.scalar.activation(
                                out=hA[i][:], in_=psA[i][:], func=Relu,
                                bias=bn2a4b_t[:], scale=bn2a4s_t[:])
                        nc.scalar.activation(
                            out=hA2[:], in_=psA2[:], func=Relu,
                            bias=bn2a4b_t[0:32, :], scale=bn2a4s_t[0:32, :])
                        # path 2 stage B
                        ft2 = ftp.tile([64, BLK], f32, tag="ft2", bufs=1)
                        nc.tensor.matmul(out=ft2[:], lhsT=w2bs_t[0][:],
                                         rhs=hA[0][:], start=True, stop=False)
                        nc.tensor.matmul(out=ft2[:], lhsT=w2bs_t[1][:],
                                         rhs=hA[1][:], start=False, stop=False)
                        nc.tensor.matmul(out=ft2[:], lhsT=w2b8_t[:],
                                         rhs=hA2[:], start=False, stop=False)
                        # subtract the spurious h(0)=relu(b) contribution of
                        # missing neighbors: ft2 += mask.T @ (-relu(b)@W2b)
                        pm = tp.tile([9, BLK], f16, tag="tp")
                        for jj in range(JB):
                            nc.tensor.transpose(
                                out=pm[:, jj * 128:(jj + 1) * 128],
                                in_=msk1[:, j0 + jj, :],
                                identity=identH[:])
                        mch = chp.tile([9, BLK], f16, tag="chm", bufs=4)
                        nc.vector.tensor_copy(out=mch[:], in_=pm[:])
                        nc.tensor.matmul(out=ft2[:], lhsT=em9_t[:],
                                         rhs=mch[:], start=False, stop=True)

                        # BN + ReLU -> fp16 ms tiles
                        msA = msp.tile([128, BLK], f16, tag="msA")
                        nc.scalar.activation(out=msA[:], in_=ft13[:], func=Relu,
                                             bias=bn13b_t[:], scale=bn13s_t[:])
                        msB = msp.tile([64, BLK], f16, tag="msB")
                        nc.scalar.activation(out=msB[:], in_=ft2[:], func=Relu,
                                             bias=bn2bb_t[:], scale=bn2bs_t[:])
                        # running max + store
                        nc.vector.tensor_reduce(
                            out=rmA[:, blk:blk + 1], in_=msA[:],
                            axis=mybir.AxisListType.X, op=mybir.AluOpType.max)
                        nc.vector.tensor_reduce(
                            out=rmB[:, blk:blk + 1], in_=msB[:],
                            axis=mybir.AxisListType.X, op=mybir.AluOpType.max)
                        nc.sync.dma_start(
                            out=msA_d[blk * 128:(blk + 1) * 128, :], in_=msA[:])
                        nc.sync.dma_start(
                            out=msB_d[blk * 64:(blk + 1) * 64, :], in_=msB[:])

            # ---------------- attention ----------------
            with tc.tile_pool(name="at", bufs=1) as at, \
                 tc.tile_pool(name="atp", bufs=1, space="PSUM") as atp:
                pA = at.tile([128, 1], f32)
                pB = at.tile([64, 1], f32)
                nc.vector.tensor_reduce(out=pA[:], in_=rmA[:],
                                        axis=mybir.AxisListType.X,
                                        op=mybir.AluOpType.max)
                nc.vector.tensor_reduce(out=pB[:], in_=rmB[:],
                                        axis=mybir.AxisListType.X,
                                        op=mybir.AluOpType.max)
                nc.sync.dma_start(
                    out=ccin[0:1, 0:128].rearrange("a c -> c a"), in_=pA[:])
                nc.sync.dma_start(
                    out=ccin[0:1, 128:192].rearrange("a c -> c a"), in_=pB[:])
                nc.gpsimd.collective_compute(
                    "AllReduce", mybir.AluOpType.max,
                    replica_groups=[list(range(n_cores))],
                    ins=[ccin[:, :]], outs=[ccout[:, :]])
                poolA = at.tile([128, 1], f32)
                poolB = at.tile([64, 1], f32)
                nc.sync.dma_start(
                    out=poolA[:], in_=ccout[0:1, 0:128].rearrange("a c -> c a"))
                nc.sync.dma_start(
                    out=poolB[:], in_=ccout[0:1, 128:192].rearrange("a c -> c a"))

                qp = atp.tile([16, 1], f32, tag="qp")
                nc.tensor.matmul(out=qp[:], lhsT=a1wA_t[:], rhs=poolA[:],
                                 start=True, stop=False)
                nc.tensor.matmul(out=qp[:], lhsT=a1wB_t[:], rhs=poolB[:],
                                 start=False, stop=True)
                qs = at.tile([16, 1], f32)
                nc.scalar.activation(out=qs[:], in_=qp[:], func=Relu,
                                     bias=a1b_t[:], scale=1.0)
                aA = atp.tile([128, 1], f32, tag="aA")
                nc.tensor.matmul(out=aA[:], lhsT=a2wA_t[:], rhs=qs[:],
                                 start=True, stop=True)
                aB = atp.tile([64, 1], f32, tag="aB")
                nc.tensor.matmul(out=aB[:], lhsT=a2wB_t[:], rhs=qs[:],
                                 start=True, stop=True)
                attnA = at.tile([128, 1], f32)
                attnB = at.tile([64, 1], f32)
                nc.scalar.activation(out=attnA[:], in_=aA[:], func=Sigmoid,
                                     bias=a2bA_t[:], scale=1.0)
                nc.scalar.activation(out=attnB[:], in_=aB[:], func=Sigmoid,
                                     bias=a2bB_t[:], scale=1.0)
                # fold attention into fusion weights
                wfa_s = at.tile([128, 64], f16)
                wfb_s = at.tile([64, 64], f16)
                nc.vector.tensor_tensor(
                    out=wfa_s[:], in0=wfa_t[:],
                    in1=attnA[:, 0:1].to_broadcast([128, 64]),
                    op=mybir.AluOpType.mult)
                nc.vector.tensor_tensor(
                    out=wfb_s[:], in0=wfb_t[:],
                    in1=attnB[:, 0:1].to_broadcast([64, 64]),
                    op=mybir.AluOpType.mult)

                # ---------------- pass 2 ----------------
                with tc.tile_pool(name="lp", bufs=3) as lp, \
                     tc.tile_pool(name="fp2", bufs=2, space="PSUM") as fp2, \
                     tc.tile_pool(name="op", bufs=2, space="PSUM") as op, \
                     tc.tile_pool(name="ou", bufs=2) as ou:
                    for sc in range(n_sc):
                        osb = ou.tile([128, JS, 64], f32, tag="osb")
                        for b in range(n_blk_sc):
                            blk = sc * n_blk_sc + b
                            mA = lp.tile([128, BLK], f16, tag="mA")
                            mB = lp.tile([64, BLK], f16, tag="mB")
                            nc.sync.dma_start(
                                out=mA[:], in_=msA_d[blk * 128:(blk + 1) * 128, :])
                            nc.sync.dma_start(
                                out=mB[:], in_=msB_d[blk * 64:(blk + 1) * 64, :])
                            psF = fp2.tile([64, BLK], f32, tag="psF")
                            nc.tensor.matmul(out=psF[:], lhsT=wfa_s[:],
                                             rhs=mA[:], start=True, stop=False)
                            nc.tensor.matmul(out=psF[:], lhsT=wfb_s[:],
                                             rhs=mB[:], start=False, stop=True)
                            fT = lp.tile([64, BLK], f32, tag="fT")
                            nc.scalar.activation(out=fT[:], in_=psF[:],
                                                 func=Relu, bias=bnfb_t[:],
                                                 scale=bnfs_t[:])
                            for jj in range(JB):
                                ot = op.tile([128, 64], f32, tag="ot")
                                nc.tensor.transpose(
                                    out=ot[:],
                                    in_=fT[:, jj * 128:(jj + 1) * 128],
                                    identity=identF[:])
                                nc.vector.tensor_copy(
                                    out=osb[:, b * JB + jj, :], in_=ot[:])
                        nc.sync.dma_start(
                            out=out_d[sc * S:(sc + 1) * S, :].rearrange(
                                "(p j) c -> p j c", p=P),
                            in_=osb[:])

    nc.compile()
    return nc


def prep_inputs(inputs, n_cores, nlp, NT):
    """Host-side input massaging: fp16 table with dummy zero row, per-core
    nbr slices (remapped/padded), packed weights. Returns per-core in_maps."""
    f = np.asarray(inputs["features"], np.float32)
    N = f.shape[0]
    nloc = N // n_cores
    ftab = np.zeros((NT, 64), np.float16)
    ftab[:N] = f.astype(np.float16)

    def prep_nbr(nbr):
        nbr = np.asarray(nbr).astype(np.int64)
        nbr = np.where(nbr < 0, N, nbr).astype(np.int32)
        outs = []
        for c in range(n_cores):
            sl = nbr[:, c * nloc:(c + 1) * nloc]
            pad = np.repeat(sl[:, :1], nlp - nloc, axis=1)
            outs.append(np.ascontiguousarray(
                np.concatenate([sl, pad], axis=1)))
        return outs

    nbr1_c = prep_nbr(inputs["nbr_d1"])
    nbr2_c = prep_nbr(inputs["nbr_d2"])

    W1 = np.asarray(inputs["W1"], np.float32)
    W2a = np.asarray(inputs["W2a"], np.float32)
    W2b = np.asarray(inputs["W2b"], np.float32)
    W3 = np.asarray(inputs["W3"], np.float32)
    Wf = np.asarray(inputs["Wf"], np.float32)
    A1w = np.asarray(inputs["A1_w"], np.float32)
    A1b = np.asarray(inputs["A1_b"], np.float32)
    A2w = np.asarray(inputs["A2_w"], np.float32)
    A2b = np.asarray(inputs["A2_b"], np.float32)

    w1p = np.concatenate([np.concatenate([W1[2 * j], W1[2 * j + 1]], axis=0)
                          for j in range(4)], axis=0).astype(np.float16)
    w3p = np.concatenate([np.concatenate([W3[2 * j], W3[2 * j + 1]], axis=0)
                          for j in range(4)], axis=0).astype(np.float16)
    w2abd = np.zeros((128, 64), np.float16)
    w2abd[0:64, 0:32] = W2a
    w2abd[64:128, 32:64] = W2a
    w2bs = np.concatenate([np.concatenate(list(W2b[0:4]), axis=0),
                           np.concatenate(list(W2b[4:8]), axis=0)],
                          axis=0).astype(np.float16)

    perm = np.r_[0:64, 128:192, 64:128]
    Wfp = Wf[perm]
    A1wp = A1w[perm]
    A2wp = A2w[:, perm]
    A2bp = A2b[perm]

    def col(x):
        return np.ascontiguousarray(x.reshape(-1, 1).astype(np.float32))

    bn13s = np.concatenate([np.asarray(inputs["bn1_s"]),
                            np.asarray(inputs["bn3_s"])])
    bn13b = np.concatenate([np.asarray(inputs["bn1_b"]),
                            np.asarray(inputs["bn3_b"])])
    base = dict(
        ftab=ftab,
        w1p=w1p, w1s=W1[8].astype(np.float16),
        w3p=w3p, w3s=W3[8].astype(np.float16),
        w2abd=w2abd, w2as=W2a.astype(np.float16),
        w2bs=w2bs, w2b8=W2b[8].astype(np.float16),
        wfa=np.ascontiguousarray(Wfp[0:128]),
        wfb=np.ascontiguousarray(Wfp[128:192]),
        a1wA=np.ascontiguousarray(A1wp[0:128]),
        a1wB=np.ascontiguousarray(A1wp[128:192]),
        a1b=col(A1b),
        a2wA=np.ascontiguousarray(A2wp[:, 0:128]),
        a2wB=np.ascontiguousarray(A2wp[:, 128:192]),
        a2bA=col(A2bp[0:128]), a2bB=col(A2bp[128:192]),
        bn13s=col(bn13s), bn13b=col(bn13b),
        bn2a4s=col(np.tile(np.asarray(inputs["bn2a_s"]), 4)),
        bn2a4b=col(np.tile(np.asarray(inputs["bn2a_b"]), 4)),
        bn2bs=col(np.asarray(inputs["bn2b_s"])),
        bn2bb=col(np.asarray(inputs["bn2b_b"])),
        bnfs=col(np.asarray(inputs["bnf_s"])),
        bnfb=col(np.asarray(inputs["bnf_b"])),
        em9=np.ascontiguousarray(
            -(np.maximum(np.asarray(inputs["bn2a_b"], np.float32), 0.0)
              @ W2b).astype(np.float16)),
    )
    f16full = f.astype(np.float16)
    in_maps = []
    for c in range(n_cores):
        m = dict(base)
        m["nbr1"] = nbr1_c[c]
        m["nbr2"] = nbr2_c[c]
        sl = f16full[c * nloc:(c + 1) * nloc]
        m["floc"] = np.ascontiguousarray(
            np.concatenate([sl, np.repeat(sl[:1], nlp - nloc, axis=0)]))
        in_maps.append(m)
    return in_maps


# full-problem configuration
N_CORES = 8
N_FULL = 300000
NT_FULL = 300008          # 8 pad rows; row 300000 is the zero dummy
S_FULL = 512              # sites per superchunk (min JS=JB; least padding)
NLP_FULL = 37888          # 74 superchunks * 512 (>= 37500, 1.0% pad)

_cache = {}


def kernel(**inputs):
    from concourse import bass_utils

    key = "full"
    if key not in _cache:
        _cache[key] = build_kernel(NT_FULL, NLP_FULL, S_FULL, N_CORES, N_FULL)
    nc = _cache[key]
    in_maps = prep_inputs(inputs, N_CORES, NLP_FULL, NT_FULL)
    res = bass_utils.run_bass_kernel_spmd(nc, in_maps, list(range(N_CORES)))
    nloc = N_FULL // N_CORES
    return np.concatenate(
        [res.results[c]["out"][:nloc] for c in range(N_CORES)], axis=0)

